# revision 72
# baseline (speedup 1.0000x reference)
# SAGAN self-attention block (nn_Attention) on 8 TRN2 NeuronCores.
#
# Reference computation per sample (C=256, H=W=64, HW=4096, C8=32, C2=128):
#   theta = w_theta @ x            (32, 4096)
#   phi   = maxpool2(w_phi @ x)    (32, 1024)
#   g     = maxpool2(w_g @ x)      (128, 1024)
#   attn  = softmax(theta.T @ phi, axis=m)          (4096, 1024)
#   o     = w_final @ (attn @ g.T).T                (256, 4096)
#   y     = sigma * o + x
#
# Sharding: data-parallel over batch B=16 -> 2 samples per core, weights
# replicated, no collectives.
#
# Design (all matmuls bf16 with fp32 PSUM accumulation).  The kernel is
# ACT(exp)-floor bound: per n-tile the scalar engine must exp 4096
# elements/partition (~4.6us in 4 ACTIVATEs); everything else is scheduled
# to hide under that chain:
#  - scores are computed TRANSPOSED (m on partitions, n free):
#      scores_T = phi.T @ theta
#    as 4-WAY ROW-TILED K=32 matmuls (tile_position=(32t,0)): theta and phi
#    both carry 4 replicated copies of their 32 channels across the 128
#    partitions (host-side rep4 weights), so the four m-chunks of a group
#    run CONCURRENTLY in the four 32-row strips of the PE array (~395ns for
#    4 matmuls, ~3x faster than the baseline's K=128 zero-padded scores).
#    Each group's 4 matmuls write the 4 quarters of TWO [128,1024] PSUM
#    tiles (4 distinct banks, as row tiling requires); the two tiles are
#    double-buffered across groups so exp(G0) overlaps the G1 matmuls and
#    the next tile's G0 -- the ACT exp chain never waits for scores.
#  - exp on ScalarE psum->sbuf bf16, NO max subtraction (|scores| < 29
#    stays well inside fp32/bf16 range).
#  - O = g.T @ exp_T accumulated over the 8 m-chunks in PSUM (128x128
#    mode); chunks 0-5 in-tile, 6-7 deferred.
#  - softmax denominators r: exp tiles are pre-summed on the otherwise-idle
#    GPSIMD (8->4->2.5 levels), leaving THREE all-ones matmuls per n-tile.
#    M=128 replicates r across all partitions, so reciprocal_approx_fast
#    runs directly on the [128,512] PSUM tile and the normalize multiplies
#    straight out of the O accumulator (one DVE op).
#  - the O(6-7)+r+reciprocal+normalize chain of tile nt is DEFERRED into
#    tile nt+1 (emitted behind its exp G0) so the PE never stalls on the
#    ACT exp or the GPSIMD pre-sum latency.
#  - y = sigma*W_f@o_norm + x as ONE matmul (sigma folded host-side) + a
#    DVE add with bf16 x during PSUM evacuation; y stored/DMA'd bf16.
#  - 2x2 maxpool as TWO InstPool ops (w-pairs then h-pairs) instead of the
#    3-op copy/max/max chain.
#  - g.T via 8 PE transposes per sample; transpose evac on DVE.  Sample 1's
#    transposes ride the exp windows of B(0) tile 7 as fillers (no
#    inter-phase PE bubble); phase A of sample 1 rides tiles 0-6 (tile nt
#    carries A(1,nt+1); A(1,0) runs right after phase A(0)); the finals
#    ride the B(1) tiles.  theta evacs: ACT in phase A (idle there), DVE
#    when running as a filler (ACT is exp-bound in B phases).
#  - startup: ident+ones ride a TINY first DMA (sync) so the PE warm-up
#    can begin ~1.5us earlier than waiting for the weight pack (scalar
#    queue); sample-0 x chunks alternate scalar/sync so both channel
#    halves of chunk q land together (~1.8us per chunk pair); sample-1 x
#    rides the GpSimd SWDGE queue alone.  64 warm-up matmuls get the HAM
#    clock gate to 8/8 right as the first projections start.
#  - last n-tile of sample 1 keeps a direct 8-matmul PSUM-accumulated r to
#    minimise the r-chain latency on the kernel tail.
#  - PSUM budget (8 banks): scores 2x[128,1024] double-buffered = 4, O
#    accumulators 2x[128,512] double-buffered = 2, 2 rotating [128,512]
#    banks shared by r, phase-A projections, finals and transposes.

import os
import sys

sys.path.insert(0, "/opt/trn_rl_repo")

import numpy as np
import ml_dtypes

BF = ml_dtypes.bfloat16

B, C, H, W = 16, 256, 64, 64
HW = H * W            # 4096
C8, C2 = C // 8, C // 2   # 32, 128
M = HW // 4           # 1024 pooled positions
NCORES = 8
SPC = B // NCORES     # samples per core = 2
NT = HW // 512        # 8 n-tiles of 512
NCH = M // 128        # 8 m-chunks of 128

_cached = {}


def _build_graph():
    from contextlib import ExitStack
    from concourse import bacc, bass, mybir, tile

    f32 = mybir.dt.float32
    bf16 = mybir.dt.bfloat16
    Exp = mybir.ActivationFunctionType.Exp
    mx = mybir.AluOpType.max
    add = mybir.AluOpType.add

    nc = bacc.Bacc("TRN2", target_bir_lowering=False, debug=False, num_devices=NCORES)

    # ---- DRAM parameters (per-core shard) ----
    xb_d = nc.dram_tensor("xb", [SPC, C, HW], bf16, kind="ExternalInput").ap()
    wup_d = nc.dram_tensor("wup", [128, 256], bf16, kind="ExternalInput").ap()
    cpack_d = nc.dram_tensor("cpack", [128, 1024], bf16,
                             kind="ExternalInput").ap()
    y_d = nc.dram_tensor("y", [SPC, C, HW], bf16, kind="ExternalOutput").ap()

    with tile.TileContext(nc) as tc, ExitStack() as ctx:
        # ---- SBUF pools ----
        consts = ctx.enter_context(tc.tile_pool(name="consts", bufs=1))
        xbpool = ctx.enter_context(tc.tile_pool(name="xb", bufs=8 * SPC))
        thpool = ctx.enter_context(tc.tile_pool(name="theta", bufs=NT * SPC))
        phpool = ctx.enter_context(tc.tile_pool(name="phi", bufs=NCH * SPC))
        gpool = ctx.enter_context(tc.tile_pool(name="g", bufs=NCH * SPC))
        gtpool = ctx.enter_context(tc.tile_pool(name="gt", bufs=8 * SPC))
        pwpool = ctx.enter_context(tc.tile_pool(name="poolw", bufs=6))
        exppool = ctx.enter_context(tc.tile_pool(name="exp", bufs=16))
        opool = ctx.enter_context(tc.tile_pool(name="oun", bufs=SPC))
        rspool = ctx.enter_context(tc.tile_pool(name="rsum", bufs=6))
        rpool = ctx.enter_context(tc.tile_pool(name="rtiles", bufs=4))
        ypool = ctx.enter_context(tc.tile_pool(name="y", bufs=6))
        # ---- PSUM pools: 4 + 2 + 2 = 8 banks ----
        big = ctx.enter_context(tc.tile_pool(name="bigps", bufs=2, space="PSUM"))
        ops = ctx.enter_context(tc.tile_pool(name="ops", bufs=2, space="PSUM"))
        half = ctx.enter_context(tc.tile_pool(name="halfps", bufs=2, space="PSUM"))

        # ---- load constants/weights ----
        # wup (ident+ones, tiny) first on sync: the PE warm-up depends only
        # on it.  cpack rides scalar BEHIND the first x chunk so that chunk
        # is not delayed (phase A needs x AND cpack; the warm-up covers
        # cpack's extra ~1.8us).
        wup = consts.tile([128, 256], bf16, tag="wup")
        nc.sync.dma_start(wup[:], wup_d[:])
        cpack = consts.tile([128, 1024], bf16, tag="cpack")
        WTH, WPH, WG, WF = 0, 256, 512, 768

        def wsl(base, c2):
            return cpack[:, base + 128 * c2:base + 128 * (c2 + 1)]

        ident = wup[:, 0:128]
        ones = wup[:, 128:256]
        dly = consts.tile([128, 8], bf16, tag="dly")

        # ---- per-sample state ----
        xb_sb = {}
        theta = {}
        phi = {}
        g_sb = {}
        gT = {}
        o_un = {}

        def emit_x_dma(s):
            # one SBUF tile per DMA chunk: dependency tracking is
            # tile-granular for DMA writes, so a consumer of columns
            # 0:512 must not share a tile with later-arriving chunks.
            xb_sb[s] = [[xbpool.tile([128, 1024], bf16, tag="xb",
                         name=f"xb_sb{s}_{c}_{q}") for q in range(4)]
                        for c in range(2)]
            for q in range(4):
                csl = slice(1024 * q, 1024 * (q + 1))
                for c2 in range(2):
                    if s == 0:
                        # alternate scalar/sync so both channel halves of
                        # chunk q land back-to-back (phase A consumes both)
                        eng = nc.scalar if c2 == q % 2 else nc.sync
                    else:
                        eng = nc.gpsimd
                    if s == 1 and c2 == 0 and q == 0:
                        # hold sample 1's stream back until sample 0's
                        # second chunk pair has landed: the three queues
                        # share ~280GB/s, and ONLY sample 0 + the weights
                        # gate the critical path (phase A)
                        nc.gpsimd.tensor_copy(dly[:],
                                              xb_sb[0][1][0][:, 0:8])
                    eng.dma_start(xb_sb[s][c2][q][:],
                                  xb_d[s, 128 * c2:128 * (c2 + 1), csl])
                if s == 0 and q == 0:
                    # cpack immediately after the first x chunk
                    nc.scalar.dma_start(cpack[:], cpack_d[:])
            # theta/phi/g are PER-CHUNK tiles so consumers wait only the
            # chunk they read (a monolithic tile made B(0,0) wait the whole
            # of phase A through the tile-granular dependency tracking).
            theta[s] = [thpool.tile([128, 512], bf16, tag="theta",
                                    name=f"theta{s}_{i}") for i in range(NT)]
            phi[s] = [phpool.tile([128, 128], bf16, tag="phi",
                                  name=f"phi{s}_{i}") for i in range(NCH)]
            g_sb[s] = [gpool.tile([128, 128], bf16, tag="g",
                                  name=f"gsb{s}_{i}") for i in range(NCH)]
            o_un[s] = opool.tile([128, HW], bf16, tag="oun", name=f"oun{s}")

        def xsl(s, c2, nt):
            # x columns [512*nt, 512*(nt+1)) of channel half c2
            return xb_sb[s][c2][nt // 2][:, 512 * (nt % 2):512 * (nt % 2 + 1)]

        def proj(s, nt, wt, ps):
            for c2 in range(2):
                nc.tensor.matmul(ps[:], wsl(wt, c2), xsl(s, c2, nt),
                                 start=(c2 == 0), stop=(c2 == 1))

        def pool2(s, nt, src_ps, dst_t, act=False):
            # 2x2 maxpool of a (128,512) psum chunk into dst[:, 128nt:...].
            # The psum chunk is evacuated CONTIGUOUSLY to sbuf bf16 first
            # (strided psum reads cost ~3x more per element), then the
            # W-pair and H-pair maxes run sbuf->sbuf in bf16.  max() is
            # exact per element.  act=True runs the evacuation on the
            # otherwise-idle ACT engine during phase A.
            fl = pwpool.tile([128, 512], bf16, tag="poolf")
            if act:
                nc.scalar.copy(fl[:], src_ps[:])
            else:
                nc.vector.tensor_copy(fl[:], src_ps[:])
            v = fl[:].rearrange("p (h w) -> p h w", h=8)
            tmp = pwpool.tile([128, 8, 32], bf16, tag="poolw")
            nc.vector.tensor_tensor(tmp[:], v[:, :, 0::2], v[:, :, 1::2], mx)
            dv = dst_t[:].rearrange("p (h w) -> p h w", h=4)
            nc.vector.tensor_tensor(dv, tmp[:, 0::2, :], tmp[:, 1::2, :], mx)

        # During phase A the O-accumulator banks are idle: rotate the
        # projection psums across the half+ops pools for a 4-deep pipeline
        # (a 2-deep rotation serializes phase A behind the evacuations).
        a_ps_i = [0]

        def a_psum(name):
            p = (half, ops)[a_ps_i[0] % 2]
            a_ps_i[0] += 1
            return p.tile([128, 512], f32,
                          tag=("half" if p is half else "o"), name=name)

        def h_psum(name):
            return half.tile([128, 512], f32, tag="half", name=name)

        def emit_A_th(s, nt, dve=False, ps=h_psum):
            # theta evac on ACT in phase A (ACT idle there); on DVE when
            # running as a B-phase filler (ACT is exp-bound there).
            nsl = slice(512 * nt, 512 * (nt + 1))
            th_ps = ps(f"thp{s}_{nt}")
            proj(s, nt, WTH, th_ps)
            if dve:
                nc.vector.tensor_copy(theta[s][nt][:], th_ps[:])
            else:
                nc.scalar.copy(theta[s][nt][:], th_ps[:])

        def emit_A_ph(s, nt, act=False, ps=h_psum):
            ph_ps = ps(f"php{s}_{nt}")
            proj(s, nt, WPH, ph_ps)
            pool2(s, nt, ph_ps, phi[s][nt], act=act)

        def emit_A_g(s, nt, act=False, ps=h_psum):
            g_ps = ps(f"gp{s}_{nt}")
            proj(s, nt, WG, g_ps)
            pool2(s, nt, g_ps, g_sb[s][nt], act=act)

        def emit_A_nt(s, nt):
            # ph evac on ACT, g evac on DVE: balances the phase-A engines
            # (ACT: th-copy + ph-evac ~1.2us/nt, DVE: g-evac + maxes ~1.7)
            emit_A_th(s, nt, ps=a_psum)
            emit_A_ph(s, nt, act=True, ps=a_psum)
            emit_A_g(s, nt, act=False, ps=a_psum)

        def emit_gT_init(s):
            gT[s] = [gtpool.tile([128, 128], bf16, tag="gt",
                                 name=f"gT{s}_{m_}") for m_ in range(NCH)]

        def emit_gT_chunk(s, mu, ps=None):
            if ps is None:
                tp_ps = half.tile([128, 128], bf16, tag="half",
                                  name=f"tp{s}_{mu}")
            else:
                p = (half, ops)[a_ps_i[0] % 2]
                a_ps_i[0] += 1
                tp_ps = p.tile([128, 128], bf16,
                               tag=("half" if p is half else "o"),
                               name=f"tp{s}_{mu}")
            nc.tensor.transpose(tp_ps[:], g_sb[s][mu][:], ident)
            nc.vector.tensor_copy(gT[s][mu][:], tp_ps[:])

        def emit_gT(s):
            # 4-deep psum rotation (half+ops) so the transpose chain is not
            # serialized behind the DVE evacuations
            emit_gT_init(s)
            for mu in range(NCH):
                emit_gT_chunk(s, mu, ps=True)

        def emit_scores_half(s, nt, grp):
            # 4 concurrent row-tiled K=32 matmuls: m-chunk mu=4*grp+t runs
            # in PE row strip t; theta/phi copies t live at partitions
            # 32t:32t+32.  The 4 matmuls write the 4 quarters of two
            # [128,1024] PSUM tiles (= 4 distinct banks, required for
            # concurrent row tiles); each tile gets its own exp so the
            # group's banks free up incrementally.
            nsl = slice(512 * nt, 512 * (nt + 1))
            ts_ = [big.tile([128, 1024], f32, tag="big",
                            name=f"sc{s}_{nt}_{grp}_{h}") for h in range(2)]
            for t in range(4):
                mu = 4 * grp + t
                nc.tensor.matmul(
                    ts_[t // 2][:, 512 * (t % 2):512 * (t % 2 + 1)],
                    phi[s][mu][32 * t:32 * (t + 1), :],
                    theta[s][nt][32 * t:32 * (t + 1), :],
                    start=True, stop=True, tile_position=(32 * t, 0))
            ets = []
            for h in range(2):
                et = exppool.tile([128, 1024], bf16, tag="exp",
                                  name=f"exp{s}_{nt}_{grp}_{h}")
                nc.scalar.activation(et[:], ts_[h][:], Exp)
                ets.append(et)
            return ets

        def emit_B_nt(s, nt, fill_w1, fill_w2, fast_tail=False, pending=None,
                      ets0=None, prefetch=None):
            """One attention n-tile.  fill_w1/fill_w2: filler callables for
            the exp-G0 / exp-G1 PE wait windows.  pending: the PREVIOUS
            tile's deferred chain (O chunks 6-7 + r matmuls + reciprocal +
            normalize), emitted behind this tile's exp G0 so its inputs
            (exp G1 prev, GPSIMD pre-sums) are certainly ready and the PE
            never stalls.  ets0: this tile's G0 exps if already emitted by
            the previous tile's prefetch; prefetch: emits the NEXT tile's
            G0 scores -- called before the last O matmuls so the ACT exp
            chain rolls straight into the next tile.  Returns (deferred
            chain, next tile's ets0)."""
            nsl = slice(512 * nt, 512 * (nt + 1))

            e0a, e0b = ets0 if ets0 is not None else emit_scores_half(s, nt, 0)
            # ---- exp-G0 window: prev tile's deferred chain, then the G1
            # scores IMMEDIATELY (they must complete inside exp-G0b's
            # shadow or the ACT exp chain stalls); fillers after ----
            if pending is not None:
                pending()
            e1a, e1b = emit_scores_half(s, nt, 1)
            # pre-sums for r: GPSIMD folds chunks 0-3 (rs0, via e0a+e0b)
            # and 4+5 (rs1a); chunks 6-7 (e1b) are streamed by the r
            # matmuls DIRECTLY -- e1b lands too late in the tile for a
            # GPSIMD fold to complete before the deferred r matmuls run
            # (it stalled the whole PE queue ~1.7us/tile).  The rs0
            # second-level fold rides DVE.
            rs0 = rspool.tile([128, 1024], bf16, tag="rs",
                              name=f"rs0_{s}_{nt}")
            nc.gpsimd.tensor_tensor(rs0[:], e0a[:], e0b[:], add)
            rsF0 = rspool.tile([128, 512], bf16, tag="rsf",
                               name=f"rsF0_{s}_{nt}")
            nc.gpsimd.tensor_tensor(rsF0[:], rs0[:, 0:512],
                                    rs0[:, 512:1024], add)
            rs1a = rspool.tile([128, 512], bf16, tag="rsf",
                               name=f"rs1a_{s}_{nt}")
            nc.gpsimd.tensor_tensor(rs1a[:], e1a[:, 0:512],
                                    e1a[:, 512:1024], add)

            for f in fill_w1:
                f()
            # ---- O chunks 0-3 (exp G0), fillers, then 4-5 (exp G1a) ----
            o_ps = ops.tile([128, 512], f32, tag="o", name=f"o{s}_{nt}")
            for q in range(4):
                et = (e0a, e0b)[q // 2]
                nc.tensor.matmul(o_ps[:], gT[s][q][:],
                                 et[:, 512 * (q % 2):512 * (q % 2 + 1)],
                                 start=(q == 0), stop=False)
            for f in fill_w2:
                f()
            ets0_next = prefetch() if prefetch is not None else None
            for q in (4, 5):
                nc.tensor.matmul(o_ps[:], gT[s][q][:],
                                 e1a[:, 512 * (q % 2):512 * (q % 2 + 1)],
                                 start=False, stop=False)

            if fast_tail:
                # inline everything; direct 8-matmul r skips the pre-sum
                # chain latency on the very last tile.
                for q in (6, 7):
                    nc.tensor.matmul(o_ps[:], gT[s][q][:],
                                     e1b[:, 512 * (q % 2):512 * (q % 2 + 1)],
                                     start=False, stop=(q == 7))
                r_ps = half.tile([128, 512], f32, tag="half",
                                 name=f"r{s}_{nt}")
                # the GPSIMD pre-sums for this tile are ready by exp-G1:
                # 5 matmuls instead of a direct 8 shortens the kernel tail
                for k, rs in enumerate((e1b[:, 0:512], e1b[:, 512:1024],
                                        rs0[:, 0:512], rs0[:, 512:1024],
                                        rs1a[:])):
                    nc.tensor.matmul(r_ps[:], ones, rs,
                                     start=(k == 0), stop=(k == 4))
                rbt = rpool.tile([128, 512], f32, tag="rb",
                                 name=f"rb{s}_{nt}")
                nc.vector.reciprocal_approx_fast(rbt[:], r_ps[:])
                nc.vector.tensor_mul(o_un[s][:, nsl], o_ps[:], rbt[:])
                return None, ets0_next

            def pending_next():
                for q in (6, 7):
                    nc.tensor.matmul(o_ps[:], gT[s][q][:],
                                     e1b[:, 512 * (q % 2):512 * (q % 2 + 1)],
                                     start=False, stop=(q == 7))
                r_ps = half.tile([128, 512], f32, tag="half",
                                 name=f"r{s}_{nt}")
                # GPSIMD-independent streams first: the rs1a fold is the
                # last pre-sum off the GPSIMD queue and stalled the PE
                # ~0.5us/tile when consumed first
                nc.tensor.matmul(r_ps[:], ones, e1b[:, 0:512],
                                 start=True, stop=False)
                nc.tensor.matmul(r_ps[:], ones, e1b[:, 512:1024],
                                 start=False, stop=False)
                nc.tensor.matmul(r_ps[:], ones, rsF0[:],
                                 start=False, stop=False)
                nc.tensor.matmul(r_ps[:], ones, rs1a[:],
                                 start=False, stop=True)
                # r_ps rows are replicated (ones matmul, M=128) -> the
                # reciprocal runs directly on the [128,512] PSUM tile.
                rbt = rpool.tile([128, 512], f32, tag="rb",
                                 name=f"rb{s}_{nt}")
                nc.vector.reciprocal_approx_fast(rbt[:], r_ps[:])
                # fused evacuate+normalize straight out of the O accumulator
                nc.vector.tensor_mul(o_un[s][:, nsl], o_ps[:], rbt[:])

            return pending_next, ets0_next

        f_ps_cache = {}

        def emit_final_oc(s, nt, oc, fkey=None):
            # one shared PSUM tile per fkey (per B-tile): the WAR dep (next
            # f-matmul waits the previous y evacuation) only serializes
            # slack-tolerant fillers.
            nsl = slice(512 * nt, 512 * (nt + 1))
            key = fkey if fkey is not None else (s, nt)
            if key not in f_ps_cache:
                f_ps_cache[key] = half.tile([128, 512], f32, tag="half",
                                            name=f"f{key}")
            f_ps = f_ps_cache[key]
            nc.tensor.matmul(f_ps[:], wsl(WF, oc), o_un[s][:, nsl],
                             start=True, stop=True)
            y_t = ypool.tile([128, 512], bf16, tag="y",
                             name=f"y{s}_{nt}_{oc}")
            nc.vector.tensor_tensor(y_t[:], f_ps[:], xsl(s, oc, nt), add)
            nc.sync.dma_start(y_d[s, 128 * oc:128 * (oc + 1), nsl], y_t[:])

        def emit_final_nt(s, nt):
            # tail-only: separate PSUM tiles per oc so the second final's
            # matmul does not WAR-wait on the first's y evacuation
            for oc in range(2):
                emit_final_oc(s, nt, oc, fkey=f"tail_{s}_{nt}_{oc}")

        # ================= program =================
        emit_x_dma(0)
        emit_x_dma(1)
        # PE warm-up on ident while x+weights land (~13us): N=512 streams
        # keep the PE continuously busy so the HAM clock gate reaches 8/8
        # before the first projection.
        wu_ps = half.tile([128, 512], f32, tag="half", name="warmup")
        for _ in range(17):
            nc.tensor.matmul(wu_ps[:, 0:256], ident, wup[:],
                             start=True, stop=True)
        # phase A with the g transposes interleaved right behind their
        # pools (a separate gT phase serialized the A->B boundary) and the
        # sample-1 theta projections folded in (the PE has slack here; it
        # is the B(0) pacer otherwise)
        emit_gT_init(0)
        for nt in range(NT):
            emit_A_nt(0, nt)
            if nt >= 2:
                # lagged 2 tiles: sample 1's x stream is deliberately held
                # back behind sample 0's, so early chunks land late
                emit_A_th(1, nt - 2, ps=a_psum)
            if nt >= 1:
                emit_gT_chunk(0, nt - 1, ps=True)
        emit_gT_chunk(0, NT - 1, ps=True)
        emit_A_th(1, NT - 2, ps=a_psum)
        emit_A_th(1, NT - 1, ps=a_psum)
        # A(1,0) ph/g inline here (its x chunk has landed); A(1,1..7)
        # ph/g ride the B(0) tiles 0..6 as fillers.
        emit_A_ph(1, 0, act=True, ps=a_psum)
        emit_A_g(1, 0, ps=a_psum)
        emit_gT_init(1)
        pending = None
        ets0 = None
        for nt in range(NT):
            if nt < NT - 1:
                w1 = [lambda n2=nt + 1: emit_A_ph(1, n2)]
                w2 = [lambda n2=nt + 1: emit_A_g(1, n2)]
                pf = None
            else:
                # sample-1 transposes ride the last B(0) tile's exp windows
                w1 = [(lambda m_=m_: emit_gT_chunk(1, m_)) for m_ in range(4)]
                w2 = [(lambda m_=m_: emit_gT_chunk(1, m_)) for m_ in range(4, 8)]
                pf = None
            pending, ets0 = emit_B_nt(0, nt, w1, w2, pending=pending,
                                      ets0=ets0, prefetch=pf)
        # B(1) with finals interleaved: sample-0 tile nt, sample-1 tile nt-1
        for nt in range(NT):
            # separate psum tiles per window so the two final pairs don't
            # WAR-serialize on one tile
            fka, fkb = f"b1_{nt}_a", f"b1_{nt}_b"
            w1 = [(lambda n2=nt, k=fka: emit_final_oc(0, n2, 0, fkey=k)),
                  (lambda n2=nt, k=fka: emit_final_oc(0, n2, 1, fkey=k))]
            w2 = []
            if nt >= 1:
                w2 = [(lambda n2=nt - 1, k=fkb: emit_final_oc(1, n2, 0, fkey=k)),
                      (lambda n2=nt - 1, k=fkb: emit_final_oc(1, n2, 1, fkey=k))]
            pf = None
            pending, ets0 = emit_B_nt(1, nt, w1, w2,
                                      fast_tail=(nt == NT - 1),
                                      pending=pending, ets0=ets0, prefetch=pf)
        emit_final_nt(1, NT - 1)

    nc.compile()
    return nc


def _prep_consts(w_theta, w_phi, w_g, w_final, sigma):
    def rep4(w):  # (32, 256) -> [2, 128, 128] = c-chunks of w.T tiled 4x
        wt = np.asarray(w).T.astype(BF)  # (256, 32)
        out = np.empty((2, 128, 128), dtype=BF)
        for c2 in range(2):
            out[c2] = np.tile(wt[128 * c2:128 * (c2 + 1)], (1, 4))
        return out

    wth = rep4(w_theta)
    # phi keeps all 4 replicated copies valid: the row-tiled scores matmuls
    # read copy t from partition strip t.
    wph = rep4(w_phi)
    wgt = np.ascontiguousarray(
        np.asarray(w_g).T.astype(BF).reshape(2, 128, 128))
    wf = (np.float32(sigma) * np.asarray(w_final)).T.astype(BF)  # (128, 256)
    wft = np.ascontiguousarray(wf.reshape(128, 2, 128).transpose(1, 0, 2))
    ident = np.eye(128, dtype=BF)
    ones = np.ones((128, 128), dtype=BF)

    def cols(w2):  # [2,128,128] -> [128, 256] with c2-major columns
        return np.concatenate([w2[0], w2[1]], axis=1)

    cpack = np.ascontiguousarray(np.concatenate(
        [cols(wth), cols(wph), cols(wgt), cols(wft)], axis=1))
    wup = np.ascontiguousarray(np.concatenate([ident, ones], axis=1))
    return dict(cpack=cpack, wup=wup)


def make_in_maps(x, w_theta, w_phi, w_g, w_final, sigma):
    consts = _prep_consts(w_theta, w_phi, w_g, w_final, sigma)
    xf = np.ascontiguousarray(np.asarray(x).reshape(B, C, HW).astype(np.float32))
    xbf = np.ascontiguousarray(xf.astype(BF))
    in_maps = []
    for core in range(NCORES):
        m = {"xb": xbf[SPC * core:SPC * (core + 1)]}
        m.update(consts)
        in_maps.append(m)
    return in_maps


def get_graph():
    if "nc" not in _cached:
        _cached["nc"] = _build_graph()
    return _cached["nc"]


def kernel(**inputs):
    from concourse.bass_utils import run_bass_kernel_spmd

    nc = get_graph()
    in_maps = make_in_maps(**inputs)
    res = run_bass_kernel_spmd(nc, in_maps, core_ids=list(range(NCORES)))
    y = np.concatenate([r["y"] for r in res.results], axis=0)
    return y.reshape(B, C, H, W).astype(np.float32)


if __name__ == "__main__":
    nc = get_graph()
    print("graph built and compiled OK")
